# revision 5
# baseline (speedup 1.0000x reference)
"""GQA kernel for Trainium2, 8-core SPMD.

Strategy (tensor-parallel over heads + sequence-parallel o_proj):
  Launch 1 (TP over heads): core c owns q-heads [4c..4c+4) and kv-head c.
    Inputs are host-transposed so every matmul contraction dim is the
    partition dim: xT [D, B*S], wqT shard [D, 256], wkvT shard [D, 128].
    Per core: QKV projections -> RoPE -> causal attention (scores computed
    transposed S^T[k,q] so softmax sums fold into the A@V matmul via a
    ones-augmented V), emits normalized attention output A^T [256, B*S].
  Host: reshard A^T head-major -> token-major (pure data movement).
  Launch 2 (sequence-parallel): core d computes o_proj for its 512 token
    rows: out = A^T.T @ woT, no cross-core reduction needed.

All matmuls run in float32r (full PE rate at free-dim >= 256, fp32 data).
The BIR verifier requires every producer feeding an f32r matmul to write
f32r-typed output, so matmul-operand tiles/DRAM tensors are declared f32r.
"""

import numpy as np
from contextlib import ExitStack

import concourse.bass as bass
import concourse.tile as tile
from concourse import bacc, mybir
from concourse.bass_utils import run_bass_kernel_spmd
from concourse.masks import make_identity

F32 = mybir.dt.float32
F32R = mybir.dt.float32r
EXP = mybir.ActivationFunctionType.Exp

B, S, D = 2, 2048, 2048
H, KVH, HD = 32, 8, 64
CORES = 8
T = B * S                    # 4096 flat tokens
HPC = H // CORES             # 4 q heads per core
QCH = HPC * HD               # 256 q rows per core
TCH = 512                    # projection t-chunk
NT = T // TCH                # 8
QB = 512                     # attention q block
NQB = S // QB                # 4 per batch
KC = 128                     # attention k chunk
TSH = T // CORES             # 512 token rows per core (launch 2)

_CACHE = {}


def _build_attn():
    nc = bacc.Bacc("TRN2", target_bir_lowering=False, debug=False,
                   num_devices=CORES)
    xT = nc.dram_tensor("xT", [D, T], F32R, kind="ExternalInput").ap()
    wqT = nc.dram_tensor("wqT", [D, QCH], F32R, kind="ExternalInput").ap()
    wkvT = nc.dram_tensor("wkvT", [D, 2 * HD], F32R, kind="ExternalInput").ap()
    cosT = nc.dram_tensor("cosT", [128, T], F32, kind="ExternalInput").ap()
    sinT = nc.dram_tensor("sinT", [128, T], F32, kind="ExternalInput").ap()
    at_out = nc.dram_tensor("at_out", [QCH, T], F32, kind="ExternalOutput").ap()

    with tile.TileContext(nc) as tc, ExitStack() as ctx:
        NJ = D // 128  # 16 contraction chunks

        const = ctx.enter_context(tc.tile_pool(name="const", bufs=1))
        wq_sb = const.tile([128, NJ, QCH], F32R, name="wq_sb")
        nc.sync.dma_start(wq_sb[:], wqT.rearrange("(jo p) i -> p jo i", p=128))
        wkv_sb = const.tile([128, NJ, 2 * HD], F32R, name="wkv_sb")
        nc.sync.dma_start(wkv_sb[:], wkvT.rearrange("(jo p) i -> p jo i", p=128))
        cos_sb = const.tile([128, T], F32, name="cos_sb")
        nc.sync.dma_start(cos_sb[:], cosT[:])
        sin_sb = const.tile([128, T], F32, name="sin_sb")
        nc.sync.dma_start(sin_sb[:], sinT[:])
        ident = const.tile([128, 128], F32, name="ident")
        make_identity(nc, ident[:])
        ones_f = const.tile([128, 1], F32, name="ones_f")
        nc.gpsimd.memset(ones_f[:], 1.0)
        ones1 = const.tile([1, 64], F32R, name="ones1")
        nc.any.tensor_copy(out=ones1[:], in_=ones_f[0:1, 0:1].to_broadcast((1, 64)))
        # diagonal-block causal masks: mask[r][kp, qf] = 1 if kp + r*128 <= qf
        masks = []
        for r in range(QB // KC):
            m = const.tile([128, QB], F32, name=f"mask{r}")
            nc.gpsimd.memset(m[:], 1.0)
            nc.gpsimd.affine_select(
                out=m[:], in_=m[:], compare_op=mybir.AluOpType.is_ge,
                fill=0.0, base=-r * KC, pattern=[[1, QB]], channel_multiplier=-1)
            masks.append(m)

        # persistent activations
        acts = ctx.enter_context(tc.tile_pool(name="acts", bufs=1))
        qt = acts.tile([128, HPC // 2, T], F32R, name="qt")
        kt = acts.tile([128, T], F32R, name="kt")
        v_aug = acts.tile([128, T // 128, HD + 1], F32R, name="v_aug")
        # col 64 = 1.0 -> the A@V matmul also emits softmax denominators
        nc.any.tensor_copy(out=v_aug[:, :, HD:HD + 1],
                           in_=ones_f[:, 0:1, None].to_broadcast((128, T // 128, 1)))

        # ---- Phase B: projections + RoPE + V transpose ----
        with ExitStack() as pctx:
            xpool = pctx.enter_context(tc.tile_pool(name="xrhs", bufs=4))
            ppool = pctx.enter_context(tc.tile_pool(name="proj_ps", bufs=3, space="PSUM"))
            tpool = pctx.enter_context(tc.tile_pool(name="rope_tmp", bufs=2))
            vps = pctx.enter_context(tc.tile_pool(name="vt_ps", bufs=2, space="PSUM"))

            for tc_i in range(NT):
                ts = slice(tc_i * TCH, (tc_i + 1) * TCH)
                ps_q = [ppool.tile([128, TCH], F32, tag="psq", name="psq")
                        for _ in range(2)]
                ps_kv = ppool.tile([128, TCH], F32, tag="pskv", name="pskv")
                for j in range(NJ):
                    rhs = xpool.tile([128, TCH], F32R, tag="rhs", name="rhs")
                    nc.sync.dma_start(rhs[:], xT[j * 128:(j + 1) * 128, ts])
                    st, sp = j == 0, j == NJ - 1
                    for ich in range(2):
                        nc.tensor.matmul(
                            ps_q[ich][:],
                            wq_sb[:, j, ich * 128:(ich + 1) * 128],
                            rhs[:], start=st, stop=sp)
                    nc.tensor.matmul(ps_kv[:], wkv_sb[:, j, :], rhs[:],
                                     start=st, stop=sp)

                # Q: copy psum -> qt, then RoPE in place
                for ich in range(2):
                    dst = qt[:, ich, ts]
                    nc.any.tensor_copy(out=dst, in_=ps_q[ich][:])
                    rot = tpool.tile([128, TCH], F32R, tag="qrot", name="qrot")
                    for hb in (0, 64):
                        nc.sync.dma_start(rot[hb:hb + 32, :], qt[hb + 32:hb + 64, ich, ts])
                        nc.sync.dma_start(rot[hb + 32:hb + 64, :], qt[hb:hb + 32, ich, ts])
                    nc.vector.tensor_mul(rot[:], rot[:], sin_sb[:, ts])
                    nc.vector.tensor_mul(dst, dst, cos_sb[:, ts])
                    nc.vector.tensor_add(dst, dst, rot[:])

                # K: rows 0:64 of kv psum -> kt, RoPE, duplicate to 64:128
                kdst = kt[0:64, ts]
                nc.any.tensor_copy(out=kdst, in_=ps_kv[0:64, :])
                krot = tpool.tile([64, TCH], F32R, tag="krot", name="krot")
                nc.sync.dma_start(krot[0:32, :], kt[32:64, ts])
                nc.sync.dma_start(krot[32:64, :], kt[0:32, ts])
                nc.vector.tensor_mul(krot[:], krot[:], sin_sb[0:64, ts])
                nc.vector.tensor_mul(kdst, kdst, cos_sb[0:64, ts])
                nc.vector.tensor_add(kdst, kdst, krot[:])
                nc.sync.dma_start(kt[64:128, ts], kt[0:64, ts])

                # V: rows 64:128 of kv psum -> sbuf, transpose 128-blocks into v_aug
                vtmp = tpool.tile([64, TCH], F32, tag="vtmp", name="vtmp")
                nc.any.tensor_copy(out=vtmp[:], in_=ps_kv[64:128, :])
                for sub in range(TCH // 128):
                    ps_t = vps.tile([128, HD], F32, tag="ps_t", name="ps_t")
                    nc.tensor.transpose(ps_t[:], vtmp[:, sub * 128:(sub + 1) * 128],
                                        ident[0:64, 0:64])
                    nc.any.tensor_copy(
                        out=v_aug[:, tc_i * (TCH // 128) + sub, 0:HD], in_=ps_t[:])

        # ---- Phase C: attention ----
        with ExitStack() as actx:
            spool = actx.enter_context(tc.tile_pool(name="sc_ps", bufs=3, space="PSUM"))
            opool = actx.enter_context(tc.tile_pool(name="o_ps", bufs=4, space="PSUM"))
            bpool = actx.enter_context(tc.tile_pool(name="bc_ps", bufs=1, space="PSUM"))
            epool = actx.enter_context(tc.tile_pool(name="exp", bufs=6))
            npool = actx.enter_context(tc.tile_pool(name="norm", bufs=4))

            for b in range(B):
                for ich in range(2):
                    for qb in range(NQB):
                        qs = slice(b * S + qb * QB, b * S + (qb + 1) * QB)
                        n_kc = (qb + 1) * (QB // KC)
                        ps_o = [opool.tile([HD + 1, QB], F32, tag="pso", name="pso")
                                for _ in range(2)]
                        for kc in range(n_kc):
                            ks = slice(b * S + kc * KC, b * S + (kc + 1) * KC)
                            st, sp = kc == 0, kc == n_kc - 1
                            for half in range(2):
                                hb = 64 * half
                                ps_s = spool.tile([128, QB], F32, tag="pss", name="pss")
                                nc.tensor.matmul(
                                    ps_s[:],
                                    kt[hb:hb + 64, ks],
                                    qt[hb:hb + 64, ich, qs],
                                    start=True, stop=True)
                                ex = epool.tile([128, QB], F32R, tag="ex", name="ex")
                                nc.scalar.activation(ex[:], ps_s[:], EXP, 0.0,
                                                     float(HD) ** -0.5)
                                r = kc - (QB // KC) * qb
                                if r >= 0:
                                    nc.vector.tensor_mul(ex[:], ex[:], masks[r][:])
                                nc.tensor.matmul(
                                    ps_o[half][:],
                                    v_aug[:, b * (S // 128) + kc, :],
                                    ex[:], start=st, stop=sp)
                        for half in range(2):
                            rec = npool.tile([1, QB], F32R, tag="rec", name="rec")
                            with nc.allow_low_precision(
                                    reason="softmax denom reciprocal feeds "
                                           "f32r broadcast matmul"):
                                nc.vector.reciprocal(rec[:], ps_o[half][HD:HD + 1, :])
                            ps_b = bpool.tile([64, QB], F32, tag="psb", name="psb")
                            nc.tensor.matmul(ps_b[:], ones1[:], rec[:],
                                             start=True, stop=True)
                            rb = npool.tile([64, QB], F32, tag="rb", name="rb")
                            nc.any.tensor_copy(out=rb[:], in_=ps_b[:])
                            ao = npool.tile([64, QB], F32, tag="ao", name="ao")
                            nc.vector.tensor_mul(ao[:], ps_o[half][0:HD, :], rb[:])
                            hl = 2 * ich + half
                            nc.sync.dma_start(at_out[hl * 64:(hl + 1) * 64, qs], ao[:])
    nc.compile()
    return nc


def _build_oproj():
    nc = bacc.Bacc("TRN2", target_bir_lowering=False, debug=False,
                   num_devices=CORES)
    at = nc.dram_tensor("at", [D, TSH], F32R, kind="ExternalInput").ap()
    woT = nc.dram_tensor("woT", [D, D], F32R, kind="ExternalInput").ap()
    out = nc.dram_tensor("out", [TSH, D], F32, kind="ExternalOutput").ap()

    NI = D // 128        # 16
    NTC = TSH // 128     # 4
    NM = D // 512        # 4
    with tile.TileContext(nc) as tc, ExitStack() as ctx:
        apool = ctx.enter_context(tc.tile_pool(name="at_sb", bufs=1))
        at_sb = apool.tile([128, NI, TSH], F32R, name="at_sb")
        nc.sync.dma_start(at_sb[:], at.rearrange("(io p) t -> p io t", p=128))
        wpool = ctx.enter_context(tc.tile_pool(name="wo_sb", bufs=4))
        ppool = ctx.enter_context(tc.tile_pool(name="ps", bufs=8, space="PSUM"))
        cpool = ctx.enter_context(tc.tile_pool(name="cp", bufs=4))
        for m in range(NM):
            ps = [ppool.tile([128, 512], F32, tag="ps", name="ps")
                  for _ in range(NTC)]
            for i in range(NI):
                w = wpool.tile([128, 512], F32R, tag="w", name="w")
                nc.sync.dma_start(w[:], woT[i * 128:(i + 1) * 128,
                                            m * 512:(m + 1) * 512])
                for t in range(NTC):
                    nc.tensor.matmul(
                        ps[t][:],
                        at_sb[:, i, t * 128:(t + 1) * 128],
                        w[:], start=i == 0, stop=i == NI - 1)
            for t in range(NTC):
                o = cpool.tile([128, 512], F32, tag="o", name="o")
                nc.any.tensor_copy(out=o[:], in_=ps[t][:])
                nc.sync.dma_start(out[t * 128:(t + 1) * 128,
                                      m * 512:(m + 1) * 512], o[:])
    nc.compile()
    return nc


def _host_prep(x, wq, wk, wv, wo, cos, sin):
    x = np.asarray(x, dtype=np.float32)
    xT = np.ascontiguousarray(x.reshape(T, D).T)                     # [D, T]
    wqT = np.ascontiguousarray(np.asarray(wq, np.float32).T)         # [D, H*HD]
    wkT = np.ascontiguousarray(np.asarray(wk, np.float32).T)         # [D, KVH*HD]
    wvT = np.ascontiguousarray(np.asarray(wv, np.float32).T)
    woT = np.ascontiguousarray(np.asarray(wo, np.float32).T)         # [D, D]

    cos2 = np.repeat(np.asarray(cos, np.float32), 2, axis=1).T       # [HD, S]
    sin2 = np.repeat(np.asarray(sin, np.float32), 2, axis=1).T
    sign = np.where(np.arange(HD)[:, None] < HD // 2,
                    np.float32(-1), np.float32(1))
    cosT = np.ascontiguousarray(
        np.tile(np.concatenate([cos2, cos2], axis=1), (2, 1)))       # [128, T]
    sinT = np.ascontiguousarray(
        np.tile(np.concatenate([sin2 * sign, sin2 * sign], axis=1), (2, 1)))
    return xT, wqT, wkT, wvT, woT, cosT, sinT


def kernel(x, wq, wk, wv, wo, cos, sin):
    xT, wqT, wkT, wvT, woT, cosT, sinT = _host_prep(x, wq, wk, wv, wo, cos, sin)

    if "attn" not in _CACHE:
        _CACHE["attn"] = _build_attn()
    if "oproj" not in _CACHE:
        _CACHE["oproj"] = _build_oproj()

    in_maps = []
    for c in range(CORES):
        in_maps.append({
            "xT": xT,
            "wqT": np.ascontiguousarray(wqT[:, c * QCH:(c + 1) * QCH]),
            "wkvT": np.ascontiguousarray(
                np.concatenate([wkT[:, c * HD:(c + 1) * HD],
                                wvT[:, c * HD:(c + 1) * HD]], axis=1)),
            "cosT": cosT,
            "sinT": sinT,
        })
    res1 = run_bass_kernel_spmd(_CACHE["attn"], in_maps,
                                core_ids=list(range(CORES)))
    at_full = np.concatenate([res1.results[c]["at_out"] for c in range(CORES)],
                             axis=0)                                  # [D, T]

    in_maps2 = []
    for c in range(CORES):
        in_maps2.append({
            "at": np.ascontiguousarray(at_full[:, c * TSH:(c + 1) * TSH]),
            "woT": woT,
        })
    res2 = run_bass_kernel_spmd(_CACHE["oproj"], in_maps2,
                                core_ids=list(range(CORES)))
    out = np.concatenate([res2.results[c]["out"] for c in range(CORES)], axis=0)
    return out.reshape(B, S, D)
